# revision 21
# baseline (speedup 1.0000x reference)
"""Canny filter Trainium2 Bass kernel (self-contained).

Row-sharded across 8 cores (128 rows of every image per core; the
batch-flattened NMS gather mixes images, so each core holds all 8 images
at its rows). Per-core "padded stack" per channel: 8 image blocks x 140
rows (6-row halos inline) x 1040 cols, tiled into 10 overlapping 128-row
tiles (stride 122). Vertical stencils: Toeplitz banded fp32 matmuls;
horizontal: shifted-rhs PSUM accumulation (13-tap 7x7 sobel-of-gaussian).
Hysteresis: bf16 tridiagonal matmuls.

Wall-clock optimizations vs the first version (the axon tunnel moves
~45 MB/s, so host<->device bytes dominate):
  - input shipped as u16 fixed-point (img*256), halving upload; the
    banded matrices are pre-scaled by 1/256 so on-chip math is unchanged
  - output bit-packed on chip: the 8 column-blocks of each row become
    the 8 bits of one u8 (edges are exactly 0/1), cutting download 32x
  - the jit'd shard_map executable is built once and cached (the stock
    run_bass_kernel_spmd path re-traces and re-uploads donated zero
    output buffers every call); zero buffers are created on-device
  - hmask (geometry constant) lives on device across calls
"""
import math
from contextlib import ExitStack

import numpy as np

import concourse.bass as bass
import concourse.bacc as bacc
import concourse.mybir as mybir
import concourse.tile as tile

mb = mybir
F32 = mb.dt.float32
BF16 = mb.dt.bfloat16
I32 = mb.dt.int32
U16 = mb.dt.uint16
U8 = mb.dt.uint8
ALU = mb.AluOpType
ACTF = mb.ActivationFunctionType

NCORES = 8
H = 1024
W = 1024
B = 8
C = 3
WP = 1040
WOFF = 8
BLK = 140
STACK = B * BLK
ROFFS = [0, 122, 244, 366, 488, 610, 732, 854, 976, 992]
CHUNKS = [(0, 512), (512, 512), (1024, 16)]
ROWS_PC = H // NCORES

QW = 128
QS = QW + 4
TW = B * QS  # 2080
TCHUNKS = [(0, 512), (512, 512), (1024, 32)]

T1, T2 = 10.0, 100.0
DIRS = [(0, 1), (1, 1), (1, 0), (1, -1), (0, -1), (-1, -1), (-1, 0), (-1, 1)]

QSCALE = 256.0  # img shipped as round(img*256) in u16


def _filters():
    g = np.exp(-0.5 * (np.arange(5) - 2.0) ** 2).astype(np.float64)
    vg = np.convolve(g, [1.0, 2.0, 1.0])
    vd = np.convolve(g, [1.0, 0.0, -1.0])
    hd_eff = np.zeros(7)
    hg_eff = np.zeros(7)
    for k in range(5):
        hd_eff[(k - 2 - 1) + 3] += g[k]
        hd_eff[(k - 2 + 1) + 3] -= g[k]
        hg_eff[(k - 2 - 1) + 3] += g[k]
        hg_eff[(k - 2) + 3] += 2 * g[k]
        hg_eff[(k - 2 + 1) + 3] += g[k]
    return g, vg, vd, hd_eff, hg_eff


def _banded(prof, n=128):
    r = (len(prof) - 1) // 2
    m = np.zeros((n, n), np.float32)
    for o in range(n):
        for j in range(-r, r + 1):
            i = o + j
            if 0 <= i < n:
                m[i, o] = prof[j + r]
    return m


def _build(nc):
    g, vg, vd, hd_eff, hg_eff = _filters()
    BVG = _banded(vg)
    BVD = _banded(vd)

    img_d = nc.dram_tensor("img", [C, STACK, W], U16, kind="ExternalInput")
    hmask_d = nc.dram_tensor("hmask", [STACK, 1], F32, kind="ExternalInput")
    packed_d = nc.dram_tensor("packed", [B, ROWS_PC, QW], U8, kind="ExternalOutput")

    BLKS = 152
    gm_scr = nc.dram_tensor("gm_scr", [B, BLKS, WP], F32, kind="Internal")
    ip_scr = nc.dram_tensor("ip_scr", [B, BLKS, WP], U16, kind="Internal")

    # scaled banded matrices for every (filter, tap) — precomputed on host.
    # 1/QSCALE folds the u16 fixed-point de-quantization into the weights.
    mats = []
    for j in range(-3, 4):
        if hd_eff[j + 3] != 0.0:
            mats.append(("x", j, np.float32(hd_eff[j + 3] / QSCALE) * BVG))
        if hg_eff[j + 3] != 0.0:
            mats.append(("y", j, np.float32(hg_eff[j + 3] / QSCALE) * BVD))
    allmats = np.stack([m for (_, _, m) in mats])  # [13,128,128]
    mats_t = nc.inline_tensor(
        np.ascontiguousarray(allmats.transpose(1, 0, 2).reshape(128, -1)), "mats"
    )  # [128, 13*128]

    wmask = np.zeros((128, WP), np.float32)
    wmask[:, WOFF : WOFF + W] = 1.0
    wmask_t = nc.inline_tensor(wmask, "wmask")
    bias4 = nc.inline_tensor(np.full((128, 1), 4.0, np.float32), "bias4")

    TRI_m = _banded([1.0, 1.0, 1.0]).astype(np.float32)
    TRI_x = np.zeros((16, 128), np.float32)
    TRI_x[0, 127] = 1.0
    TRI_xa = np.zeros((128, 16), np.float32)
    TRI_xb = np.zeros((16, 16), np.float32)
    for m2 in range(4):
        qq = 125 + m2
        for j in (-1, 0, 1):
            src = qq + j
            if src <= 125:
                if 0 <= src + 2 < 128:
                    TRI_xa[src + 2, m2] = 1.0
            else:
                if 0 <= src - 126 < 4:
                    TRI_xb[src - 126, m2] = 1.0
    import ml_dtypes
    tri_m_t = nc.inline_tensor(TRI_m.astype(ml_dtypes.bfloat16), "tri_m")
    tri_x_t = nc.inline_tensor(TRI_x.astype(ml_dtypes.bfloat16), "tri_x")
    tri_xa_t = nc.inline_tensor(TRI_xa.astype(ml_dtypes.bfloat16), "tri_xa")
    tri_xb_t = nc.inline_tensor(TRI_xb.astype(ml_dtypes.bfloat16), "tri_xb")

    with tile.TileContext(nc) as tc:
        with ExitStack() as octx:
            cpool = octx.enter_context(tc.tile_pool(name="consts", bufs=1))
            bias4_s = cpool.tile([128, 1], F32)
            nc.sync.dma_start(bias4_s[:], bias4.ap())

            # ---------------- stage 1 ----------------
            with ExitStack() as ctx:
                c1p = ctx.enter_context(tc.tile_pool(name="c1", bufs=1))
                inp = ctx.enter_context(tc.tile_pool(name="inp", bufs=2))
                work = ctx.enter_context(tc.tile_pool(name="work", bufs=1))
                small = ctx.enter_context(tc.tile_pool(name="small", bufs=2))
                psum = ctx.enter_context(
                    tc.tile_pool(name="psum", bufs=4, space="PSUM")
                )

                mats_s = c1p.tile([128, 13 * 128], F32)
                nc.sync.dma_start(mats_s[:], mats_t.ap())
                wmask_s = c1p.tile([128, WP], F32)
                nc.sync.dma_start(wmask_s[:], wmask_t.ap())
                hmask_s = c1p.tile([128, 10], F32)
                for t in range(10):
                    nc.sync.dma_start(
                        hmask_s[:, t : t + 1],
                        hmask_d[ROFFS[t] : ROFFS[t] + 128, :],
                    )

                def mat_ap(i):
                    return mats_s[:, 128 * i : 128 * (i + 1)]

                for t in range(10):
                    r0 = ROFFS[t]
                    gm = work.tile([128, WP], F32, tag="gm")
                    osum = work.tile([128, WP], F32, tag="osum")
                    suacc = work.tile([128, WP], F32, tag="suacc")
                    for c in range(C):
                        # ship only the 1024 real columns; zero the 8-col
                        # borders on-chip (the stencils read into them)
                        xin_u = inp.tile([128, WP], U16, tag="xinu")
                        nc.vector.memset(xin_u[:, 0:WOFF], 0)
                        nc.vector.memset(xin_u[:, WOFF + W : WP], 0)
                        nc.sync.dma_start(
                            xin_u[:, WOFF : WOFF + W], img_d[c, r0 : r0 + 128, :]
                        )
                        xin = inp.tile([128, WP], F32, tag="xin")
                        nc.vector.tensor_copy(xin[:], xin_u[:])
                        for (lo, n) in CHUNKS:
                            gxp = psum.tile([128, 512], F32, tag="gxp")
                            gyp = psum.tile([128, 512], F32, tag="gyp")
                            fx, fy = True, True
                            lastx = max(i for i, m in enumerate(mats) if m[0] == "x")
                            lasty = max(i for i, m in enumerate(mats) if m[0] == "y")
                            for mi, (kind, j, _) in enumerate(mats):
                                s, e = lo + j, lo + j + n
                                sc, ec = max(0, s), min(WP, e)
                                dst = (gxp if kind == "x" else gyp)[
                                    :, sc - s : n - (e - ec)
                                ]
                                nc.tensor.matmul(
                                    dst,
                                    mat_ap(mi),
                                    xin[:, sc:ec],
                                    start=(fx if kind == "x" else fy),
                                    stop=(mi == (lastx if kind == "x" else lasty)),
                                )
                                if kind == "x":
                                    fx = False
                                else:
                                    fy = False

                            sl = slice(lo, lo + n)
                            p2 = small.tile([128, 512], F32, tag="p2")
                            nc.scalar.square(p2[:, :n], gxp[:, :n])
                            q2 = small.tile([128, 512], F32, tag="q2")
                            nc.scalar.square(q2[:, :n], gyp[:, :n])
                            ss = small.tile([128, 512], F32, tag="ss")
                            nc.vector.tensor_tensor(
                                out=ss[:, :n], in0=p2[:, :n], in1=q2[:, :n],
                                op=ALU.add,
                            )
                            if c == 0:
                                nc.scalar.sqrt(gm[:, sl], ss[:, :n])
                            else:
                                rr = small.tile([128, 512], F32, tag="rr")
                                nc.scalar.sqrt(rr[:, :n], ss[:, :n])
                                nc.vector.tensor_tensor(
                                    out=gm[:, sl], in0=gm[:, sl],
                                    in1=rr[:, :n], op=ALU.add,
                                )
                            rc = small.tile([128, 512], F32, tag="rc")
                            nc.vector.reciprocal(rc[:, :n], gxp[:, :n])
                            qr = small.tile([128, 512], F32, tag="qr")
                            nc.vector.scalar_tensor_tensor(
                                out=qr[:, :n], in0=rc[:, :n], scalar=1.0,
                                in1=gyp[:, :n], op0=ALU.mult, op1=ALU.mult,
                            )
                            a0 = small.tile([128, 512], F32, tag="a0")
                            nc.scalar.activation(a0[:, :n], qr[:, :n], ACTF.Arctan)
                            su = small.tile([128, 512], F32, tag="su")
                            nc.vector.tensor_scalar(
                                out=su[:, :n], in0=gxp[:, :n], scalar1=0.0,
                                scalar2=None, op0=ALU.is_lt,
                            )
                            if c == 0:
                                nc.vector.tensor_copy(osum[:, sl], a0[:, :n])
                                nc.vector.tensor_copy(suacc[:, sl], su[:, :n])
                            else:
                                nc.vector.tensor_tensor(
                                    out=osum[:, sl], in0=osum[:, sl],
                                    in1=a0[:, :n], op=ALU.add,
                                )
                                nc.vector.tensor_tensor(
                                    out=suacc[:, sl], in0=suacc[:, sl],
                                    in1=su[:, :n], op=ALU.add,
                                )

                    gmm = work.tile([128, WP], F32, tag="gmm")
                    nc.vector.scalar_tensor_tensor(
                        out=gmm[:], in0=gm[:], scalar=hmask_s[:, t : t + 1],
                        in1=wmask_s[:], op0=ALU.mult, op1=ALU.mult,
                    )
                    zs = work.tile([128, WP], F32, tag="zs")
                    nc.scalar.activation(
                        zs[:], osum[:], ACTF.Identity, bias=bias4_s[:, 0:1],
                        scale=float(4.0 / math.pi),
                    )
                    z2 = work.tile([128, WP], F32, tag="z2")
                    nc.vector.scalar_tensor_tensor(
                        out=z2[:], in0=suacc[:], scalar=4.0, in1=zs[:],
                        op0=ALU.mult, op1=ALU.add,
                    )
                    zi = work.tile([128, WP], I32, tag="zi")
                    nc.vector.tensor_copy(zi[:], z2[:])
                    zm = work.tile([128, WP], I32, tag="zm")
                    nc.vector.tensor_scalar(
                        out=zm[:], in0=zi[:], scalar1=7, scalar2=None,
                        op0=ALU.bitwise_and,
                    )
                    ip16 = work.tile([128, WP], U16, tag="ip16")
                    nc.vector.tensor_copy(ip16[:], zm[:])

                    lo_r, hi_r = r0 + 3, r0 + 125
                    b0, b1 = lo_r // BLK, (hi_r - 1) // BLK
                    segs = [(lo_r, hi_r)] if b0 == b1 else [
                        (lo_r, (b0 + 1) * BLK), ((b0 + 1) * BLK, hi_r)]
                    for (s0, s1) in segs:
                        bb = s0 // BLK
                        pr0, pr1 = s0 - bb * BLK, s1 - bb * BLK
                        nc.sync.dma_start(
                            gm_scr[bb, pr0:pr1, :], gmm[s0 - r0 : s1 - r0, :]
                        )
                        nc.sync.dma_start(
                            ip_scr[bb, pr0:pr1, :], ip16[s0 - r0 : s1 - r0, :]
                        )

            # ---------------- stage 2: tail ----------------
            with ExitStack() as ctx:
                c2p = ctx.enter_context(tc.tile_pool(name="c2", bufs=1))
                tp = ctx.enter_context(tc.tile_pool(name="tail", bufs=1))
                tps = ctx.enter_context(
                    tc.tile_pool(name="tailps", bufs=2, space="PSUM")
                )

                tri_m_s = c2p.tile([128, 128], BF16)
                nc.sync.dma_start(tri_m_s[:], tri_m_t.ap())
                tri_x_s = c2p.tile([16, 128], BF16)
                nc.sync.dma_start(tri_x_s[:], tri_x_t.ap())
                tri_xa_s = c2p.tile([128, 16], BF16)
                nc.sync.dma_start(tri_xa_s[:], tri_xa_t.ap())
                tri_xb_s = c2p.tile([16, 16], BF16)
                nc.sync.dma_start(tri_xb_s[:], tri_xb_t.ap())

                # bit-pack accumulators: bit Q of acc = edge in column-block Q
                accm = c2p.tile([128, TW], F32)
                accx = c2p.tile([16, TW], F32)

                for Q in range(8):
                    wp0 = WOFF + QW * Q - 2
                    gmi = {}
                    for v, dh in (("u", -1), ("c", 0), ("d", 1)):
                        gmain = tp.tile([128, TW], F32, tag=f"gmi{v}")
                        gx_ = tp.tile([16, TW], F32, tag=f"gmix{v}")
                        for bb in range(B):
                            nc.sync.dma_start(
                                gmain[:, QS * bb : QS * bb + QS],
                                gm_scr[bb, 4 + dh : 132 + dh, wp0 : wp0 + QS],
                            )
                            nc.sync.dma_start(
                                gx_[:, QS * bb : QS * bb + QS],
                                gm_scr[bb, 132 + dh : 148 + dh, wp0 : wp0 + QS],
                            )
                        gmi[v] = (gmain, gx_)
                    ipt_m = tp.tile([128, TW], U16, tag="iptm")
                    ipt_x = tp.tile([16, TW], U16, tag="iptx")
                    for bb in range(B):
                        nc.sync.dma_start(
                            ipt_m[:, QS * bb : QS * bb + QS],
                            ip_scr[bb, 4:132, wp0 : wp0 + QS],
                        )
                        nc.sync.dma_start(
                            ipt_x[:, QS * bb : QS * bb + QS],
                            ip_scr[bb, 132:148, wp0 : wp0 + QS],
                        )

                    def tail_chain(P, sfx, ipt, gset):
                        # masks from 2 low bits of idx (pair symmetry: only
                        # i+ mod 4 selects among pair-AND planes)
                        b0m = tp.tile([P, TW], U16, tag=f"ia{sfx}")
                        nc.vector.tensor_scalar(
                            out=b0m[:], in0=ipt[:], scalar1=1, scalar2=None,
                            op0=ALU.bitwise_and,
                        )
                        b1m = tp.tile([P, TW], U16, tag=f"ib{sfx}")
                        nc.vector.tensor_scalar(
                            out=b1m[:], in0=ipt[:], scalar1=1, scalar2=1,
                            op0=ALU.logical_shift_right, op1=ALU.bitwise_and,
                        )
                        gc, gu, gd = gset["c"], gset["u"], gset["d"]
                        ismax = tp.tile([P, TW], F32, tag=f"v1{sfx}")
                        ph = tp.tile([P, 4 * QS], F32, tag=f"v2{sfx}")
                        dd = tp.tile([P, TW], F32, tag=f"v3{sfx}")
                        for bb in range(B):
                            dh, dw = DIRS[bb]
                            var = gc if dh == 0 else (gd if dh == 1 else gu)
                            # D = GM > shift(GM): valid except block-edge slots
                            lo2 = max(0, -dw)
                            hi2 = TW - max(0, dw)
                            nc.vector.tensor_tensor(
                                out=dd[:, lo2:hi2], in0=gc[:, lo2:hi2],
                                in1=var[:, lo2 + dw : hi2 + dw], op=ALU.is_gt,
                            )
                            # pair AND: P[blk j] = D[blk j] * D[blk j+4], j<4
                            nc.vector.tensor_tensor(
                                out=ph[:], in0=dd[:, 0 : 4 * QS],
                                in1=dd[:, 4 * QS : 8 * QS], op=ALU.mult,
                            )
                            # 4-way select by (bit1, bit0) of idx at block bb
                            bsl = slice(QS * bb, QS * bb + QS)
                            ta = tp.tile([P, QS], F32, tag=f"ic{sfx}")
                            nc.vector.select(
                                ta[:], b0m[:, bsl], ph[:, QS : 2 * QS],
                                ph[:, 0:QS],
                            )
                            tb = tp.tile([P, QS], F32, tag=f"id{sfx}")
                            nc.vector.select(
                                tb[:], b0m[:, bsl], ph[:, 3 * QS : 4 * QS],
                                ph[:, 2 * QS : 3 * QS],
                            )
                            nc.vector.select(
                                ismax[:, bsl], b1m[:, bsl], tb[:], ta[:]
                            )
                        thin = tp.tile([P, TW], F32, tag=f"w4{sfx}")
                        nc.vector.tensor_tensor(
                            out=thin[:], in0=ismax[:], in1=gc[:], op=ALU.mult
                        )
                        return thin

                    thin_m = tail_chain(128, "m", ipt_m,
                                        {k: v[0] for k, v in gmi.items()})
                    thin_x = tail_chain(16, "x", ipt_x,
                                        {k: v[1] for k, v in gmi.items()})

                    high_m = tp.tile([128, TW], BF16, tag="highm")
                    nc.vector.tensor_scalar(
                        out=high_m[:], in0=thin_m[:], scalar1=T2, scalar2=None,
                        op0=ALU.is_gt,
                    )
                    high_x = tp.tile([16, TW], BF16, tag="highx")
                    nc.vector.tensor_scalar(
                        out=high_x[:], in0=thin_x[:], scalar1=T2, scalar2=None,
                        op0=ALU.is_gt,
                    )
                    vs_m = tp.tile([128, TW], F32, tag="w5m")
                    vs_x = tp.tile([16, TW], F32, tag="w5x")
                    for (lo, n) in TCHUNKS:
                        ps1 = tps.tile([128, 512], F32, tag="ps1")
                        nc.tensor.matmul(
                            ps1[:, :n], tri_m_s[:], high_m[:, lo : lo + n],
                            start=True, stop=False,
                        )
                        nc.tensor.matmul(
                            ps1[:, :n], tri_x_s[:], high_x[:, lo : lo + n],
                            start=False, stop=True,
                        )
                        nc.scalar.copy(vs_m[:, lo : lo + n], ps1[:, :n])
                        ps2 = tps.tile([16, 512], F32, tag="ps2")
                        nc.tensor.matmul(
                            ps2[:, :n], tri_xa_s[:], high_m[:, lo : lo + n],
                            start=True, stop=False,
                        )
                        nc.tensor.matmul(
                            ps2[:, :n], tri_xb_s[:], high_x[:, lo : lo + n],
                            start=False, stop=True,
                        )
                        nc.scalar.copy(vs_x[:, lo : lo + n], ps2[:, :n])

                    def finish(P, sfx, vs, thin, high):
                        h3 = tp.tile([P, TW], F32, tag=f"v2{sfx}")
                        nc.vector.tensor_tensor(
                            out=h3[:, 1 : TW - 1], in0=vs[:, 0 : TW - 2],
                            in1=vs[:, 2:TW], op=ALU.add,
                        )
                        c1t = tp.tile([P, TW], F32, tag=f"v3{sfx}")
                        nc.vector.tensor_tensor(
                            out=c1t[:, 1 : TW - 1], in0=h3[:, 1 : TW - 1],
                            in1=vs[:, 1 : TW - 1], op=ALU.add,
                        )
                        highf = tp.tile([P, TW], F32, tag=f"v4{sfx}")
                        nc.vector.tensor_copy(highf[:], high[:])
                        crgt = tp.tile([P, TW], F32, tag=f"w3{sfx}")
                        nc.vector.tensor_tensor(
                            out=crgt[:, 1 : TW - 1], in0=c1t[:, 1 : TW - 1],
                            in1=highf[:, 1 : TW - 1], op=ALU.is_gt,
                        )
                        m1 = tp.tile([P, TW], F32, tag=f"v1{sfx}")
                        nc.vector.tensor_scalar(
                            out=m1[:], in0=thin[:], scalar1=T1, scalar2=None,
                            op0=ALU.is_ge,
                        )
                        m2t = tp.tile([P, TW], F32, tag=f"w1{sfx}")
                        nc.vector.tensor_scalar(
                            out=m2t[:], in0=thin[:], scalar1=T2, scalar2=None,
                            op0=ALU.is_le,
                        )
                        mm_ = tp.tile([P, TW], F32, tag=f"w2{sfx}")
                        nc.vector.tensor_tensor(
                            out=mm_[:], in0=m1[:], in1=m2t[:], op=ALU.mult
                        )
                        t_ = tp.tile([P, TW], F32, tag=f"v2{sfx}")
                        nc.vector.tensor_tensor(
                            out=t_[:, 1 : TW - 1], in0=mm_[:, 1 : TW - 1],
                            in1=crgt[:, 1 : TW - 1], op=ALU.mult,
                        )
                        ed = tp.tile([P, TW], F32, tag=f"v3{sfx}")
                        nc.vector.tensor_tensor(
                            out=ed[:, 1 : TW - 1], in0=highf[:, 1 : TW - 1],
                            in1=t_[:, 1 : TW - 1], op=ALU.add,
                        )
                        return ed

                    ed_m = finish(128, "m", vs_m, thin_m, high_m)
                    ed_x = finish(16, "x", vs_x, thin_x, high_x)

                    # pack: acc += ed * 2^Q (cols 1..TW-1 are the valid span)
                    if Q == 0:
                        nc.vector.tensor_copy(
                            accm[:, 1 : TW - 1], ed_m[:, 1 : TW - 1]
                        )
                        nc.vector.tensor_copy(
                            accx[:, 1 : TW - 1], ed_x[:, 1 : TW - 1]
                        )
                    else:
                        nc.vector.scalar_tensor_tensor(
                            out=accm[:, 1 : TW - 1], in0=ed_m[:, 1 : TW - 1],
                            scalar=float(1 << Q), in1=accm[:, 1 : TW - 1],
                            op0=ALU.mult, op1=ALU.add,
                        )
                        nc.vector.scalar_tensor_tensor(
                            out=accx[:, 1 : TW - 1], in0=ed_x[:, 1 : TW - 1],
                            scalar=float(1 << Q), in1=accx[:, 1 : TW - 1],
                            op0=ALU.mult, op1=ALU.add,
                        )

                accm8 = c2p.tile([128, TW], U8)
                nc.vector.tensor_copy(accm8[:, 1 : TW - 1], accm[:, 1 : TW - 1])
                accx8 = c2p.tile([16, TW], U8)
                nc.vector.tensor_copy(accx8[:, 1 : TW - 1], accx[:, 1 : TW - 1])
                for bb in range(B):
                    nc.sync.dma_start(
                        packed_d[bb, 0:126, :],
                        accm8[2:128, QS * bb + 2 : QS * bb + 2 + QW],
                    )
                    nc.sync.dma_start(
                        packed_d[bb, 126:128, :],
                        accx8[0:2, QS * bb + 2 : QS * bb + 2 + QW],
                    )


_STATE = {}


def _init():
    import jax
    import jax.numpy as jnp
    from jax.sharding import Mesh, PartitionSpec, NamedSharding
    from jax.experimental.shard_map import shard_map
    from concourse import bass2jax

    nc = bacc.Bacc("TRN2", target_bir_lowering=False, debug=False,
                   num_devices=NCORES)
    _build(nc)
    nc.finalize()

    bass2jax.install_neuronx_cc_hook()
    partition_name = (
        nc.partition_id_tensor.name if nc.partition_id_tensor else None
    )

    in_names, out_names, out_avals = [], [], []
    for alloc in nc.m.functions[0].allocations:
        if not isinstance(alloc, mybir.MemoryLocationSet):
            continue
        name = alloc.memorylocations[0].name
        if alloc.kind == "ExternalInput":
            if name != partition_name:
                in_names.append(name)
        elif alloc.kind == "ExternalOutput":
            out_names.append(name)
            shape = tuple(alloc.tensor_shape)
            dtype = mybir.dt.np(alloc.dtype)
            out_avals.append(jax.core.ShapedArray(shape, dtype))
    n_params = len(in_names)
    n_outs = len(out_names)
    all_in_names = list(in_names) + list(out_names)
    if partition_name is not None:
        all_in_names.append(partition_name)
    donate = tuple(range(n_params, n_params + n_outs))

    def _body(*args):
        operands = list(args)
        if partition_name is not None:
            operands.append(bass2jax.partition_id_tensor())
        outs = bass2jax._bass_exec_p.bind(
            *operands,
            out_avals=tuple(out_avals),
            in_names=tuple(all_in_names),
            out_names=tuple(out_names),
            lowering_input_output_aliases=(),
            sim_require_finite=True,
            sim_require_nnan=True,
            nc=nc,
        )
        return tuple(outs)

    devices = jax.devices()[:NCORES]
    assert len(devices) == NCORES
    mesh = Mesh(np.asarray(devices), ("core",))
    in_specs = (PartitionSpec("core"),) * (n_params + n_outs)
    out_specs = (PartitionSpec("core"),) * n_outs
    sharded = jax.jit(
        shard_map(_body, mesh=mesh, in_specs=in_specs, out_specs=out_specs,
                  check_rep=False),
        donate_argnums=donate,
        keep_unused=True,
    )
    rowsh = NamedSharding(mesh, PartitionSpec("core"))

    zshapes = [
        (NCORES * av.shape[0], *av.shape[1:]) for av in out_avals
    ]
    zdtypes = [av.dtype for av in out_avals]
    n_zouts = len(zshapes)

    # batch several donated-zero sets per execute RPC (RPCs cost ~80ms
    # and serialize); optimization_barrier keeps the identical zeros
    # from being CSE'd into one shared (thus double-donated) buffer
    NZSETS = 32

    def _zf():
        zs = tuple(
            jnp.zeros(s, d)
            for _ in range(NZSETS)
            for s, d in zip(zshapes, zdtypes)
        )
        return jax.lax.optimization_barrier(zs)

    zeros_fn = jax.jit(_zf, out_shardings=rowsh)

    # hmask: geometry constant — resident on device across calls
    hm = np.zeros((NCORES, STACK, 1), np.float32)
    for core in range(NCORES):
        r0 = ROWS_PC * core
        for pr in range(BLK):
            gr = r0 + pr - 6
            v = 1.0 if 0 <= gr < H else 0.0
            for b in range(B):
                hm[core, b * BLK + pr, 0] = v
    hmask_dev = jax.device_put(hm.reshape(NCORES * STACK, 1), rowsh)
    hmask_dev.block_until_ready()

    # persistent host staging buffer (halo borders stay zero between calls)
    big = np.zeros((NCORES, C, B, BLK, W), np.uint16)

    from concurrent.futures import ThreadPoolExecutor
    pool = ThreadPoolExecutor(max_workers=8)

    _STATE.update(
        nc=nc, sharded=sharded, zeros_fn=zeros_fn, hmask_dev=hmask_dev,
        in_names=in_names, big=big, pool=pool,
        zpool=[], n_zouts=n_zouts,
    )
    return _STATE


def kernel(img: np.ndarray) -> np.ndarray:
    st = _STATE if _STATE else _init()
    img = np.ascontiguousarray(img, dtype=np.float32)
    assert img.shape == (B, C, H, W)

    # donated zero output buffers: pop from the pool refilled (8 sets
    # per execute RPC) during earlier calls' upload windows — RPCs cost
    # ~80ms each and serialize, so creating them inline stalls the upload
    zpool = st["zpool"]
    if len(zpool) < st["n_zouts"]:
        zpool.extend(st["zeros_fn"]())
    zeros = [zpool.pop(0) for _ in range(st["n_zouts"])]

    # quantize to u16 fixed point in ONE fused pass per core:
    # floor(img*256) — truncation instead of rounding is fine because
    # the constant -1/512 bias cancels through the zero-sum derivative
    # filters, leaving the same +-1/512 noise as round-to-nearest.
    # numpy ufuncs release the GIL so this threads across cores.
    big, pool = st["big"], st["pool"]

    def _prep(core):
        r0 = ROWS_PC * core
        lo, hi = max(0, r0 - 6), min(H, r0 + 134)
        np.multiply(
            img[:, :, lo:hi, :].transpose(1, 0, 2, 3), QSCALE,
            out=big[core, :, :, lo - (r0 - 6) : hi - (r0 - 6), :],
            casting="unsafe",
        )

    list(pool.map(_prep, range(NCORES)))

    arg_map = {
        "img": big.reshape(NCORES * C, STACK, W),
        "hmask": st["hmask_dev"],
    }
    args = [arg_map[n] for n in st["in_names"]] + zeros
    outs = st["sharded"](*args)
    # enqueue d2h immediately so the fetch round trip overlaps the
    # upload/exec pipeline instead of paying its ~80ms RTT afterwards
    shards = sorted(
        outs[0].addressable_shards, key=lambda s: s.index[0].start or 0
    )
    datas = [s.data for s in shards]
    for d in datas:
        d.copy_to_host_async()
    # refill the zero pool (amortized: one RPC per NZSETS calls),
    # enqueued AFTER the d2h so it cannot delay our fetch in the
    # server's RPC order
    if len(zpool) < 2 * st["n_zouts"]:
        zpool.extend(st["zeros_fn"]())

    out = np.empty((B, 1, H, W), np.float32)

    def _fetch_unpack(core):
        pk = np.asarray(datas[core])  # [B, ROWS_PC, QW] u8
        bits = np.unpackbits(
            pk.reshape(B, ROWS_PC, QW, 1), axis=3, bitorder="little"
        )  # [b, r, c, Q]
        out[:, 0, ROWS_PC * core : ROWS_PC * (core + 1)] = (
            bits.transpose(0, 1, 3, 2).reshape(B, ROWS_PC, W)
        )

    list(pool.map(_fetch_unpack, range(NCORES)))
    out[..., 0, :] = 0.0
    out[..., -1, :] = 0.0
    out[..., :, 0] = 0.0
    out[..., :, -1] = 0.0
    return out


if __name__ == "__main__":
    rng = np.random.RandomState(0)
    x = (rng.rand(B, C, H, W) * 255).astype(np.float32)
    y = kernel(x)
    print("out", y.shape, y.mean())
